# revision 9
# baseline (speedup 1.0000x reference)
"""Trainium2 Bass kernel for the SEIAR neural-ODE (Tsit5, 1023 intervals x 8 substeps).

Algorithm (everything on-device, replicated on 8 cores):
  Phase A: batched MLP evaluates beta(t) at all 1024*48 stage times (PE matmuls
           + ACT softplus/sigmoid).
  Phase B: parallel-in-time Newton. All 1023 intervals are advanced
           simultaneously; each iteration evaluates the interval map Phi and its
           Jacobian J (via 5 tangent columns carried alongside the state), forms
           the residual c_n = Phi(Z[n-1]) - Z[n], and solves the linearized
           block-bidiagonal system with a hierarchical affine scan
           (free-dim Hillis-Steele + PE shift-matmul partition scan).
           RK2 coarse iterations build the initial guess, then full Tsit5
           Newton iterations converge to ~1e-5 relative of the sequential fp32
           rollout.

Interval m = p*8 + q  (p = partition 0..127, q = 0..7); m = 1023 is padding.
"""

import sys

sys.path.insert(0, "/opt/trn_rl_repo")

import numpy as np

import concourse.bacc as bacc
import concourse.mybir as mybir
from concourse.tile import TileContext
from concourse.bass_utils import run_bass_kernel_spmd

F32 = mybir.dt.float32
AL = mybir.AluOpType
ACTF = mybir.ActivationFunctionType

f32 = np.float32

SUBSTEPS = 8
C2, C3, C4, C5, C6 = 0.161, 0.327, 0.9, 0.9800255409045097, 1.0
A_TAB = [
    [],
    [0.161],
    [-0.008480655492356989, 0.335480655492357],
    [2.8971530571054935, -6.359448489975075, 4.3622954328695815],
    [5.325864828439257, -11.748883564062828, 7.4955393428898365, -0.09249506636175525],
    [5.86145544294642, -12.92096931784711, 8.159367898576159, -0.071584973281401,
     -0.028269050394068383],
]
B_TAB = [0.09646076681806523, 0.01, 0.4798896504144996, 1.379008574103742,
         -3.290069515436081, 2.324710524099774]
CS = [0.0, C2, C3, C4, C5, C6]
KK, AA, II, P_, F_, EE, DD, Q_ = 0.526, 0.244, 0.244, 0.667, 0.98, 0.0, 1.0, 0.5

KKf = f32(KK)
PKK = f32(np.float64(P_) * np.float64(KK))
QKK = f32(np.float64(1.0 - P_) * np.float64(KK))
AAf = f32(AA)
IIf = f32(II)
FAA = f32(np.float64(F_) * np.float64(AA))

H = f32(0.125)

N_COARSE = 8
N_FINE = 3
CLAMP = 1e30
BOX_LO = -0.5
BOX_HI = 1.5

P = 128
Q = 8
M = P * Q          # 1024 padded intervals (1023 real)
NT = M * 48
NCOLS = 6          # state + 5 tangents
NC_ = 5            # components S,E,I,A,R

_CACHE = {}


def _f(x):
    return float(f32(x))


def _hA(j, l):
    return float(f32(H * f32(A_TAB[j][l])))


def _build_program(sim_no_collective=False):
    from contextlib import ExitStack

    nc = bacc.Bacc("TRN2", target_bir_lowering=False, num_devices=8)

    t_stage_d = nc.dram_tensor("t_stage", [1, NT // 8], F32, kind="ExternalInput")
    w_in_d = nc.dram_tensor("w_in_t", [1, 64], F32, kind="ExternalInput")
    b_in_d = nc.dram_tensor("b_in_v", [64, 1], F32, kind="ExternalInput")
    w_h_d = nc.dram_tensor("w_h_t", [64, 64], F32, kind="ExternalInput")
    b_h_d = nc.dram_tensor("b_h_v", [64, 1], F32, kind="ExternalInput")
    w_out_d = nc.dram_tensor("w_out_t", [64, 1], F32, kind="ExternalInput")
    sigb_d = nc.dram_tensor("sig_bias", [P, 1], F32, kind="ExternalInput")
    z0_d = nc.dram_tensor("z0_row", [1, NC_], F32, kind="ExternalInput")
    winit_d = nc.dram_tensor("w_init", [P, Q, NC_], F32, kind="ExternalInput")
    tang_d = nc.dram_tensor("tang_init", [P, Q * NCOLS * NC_], F32, kind="ExternalInput")
    shifts_d = nc.dram_tensor("shifts", [7, P, P], F32, kind="ExternalInput")
    idpat_d = nc.dram_tensor("idpat", [1, NC_ * NCOLS], F32, kind="ExternalInput")
    sel_d = nc.dram_tensor("sel", [7, 1, P], F32, kind="ExternalInput")

    out_d = nc.dram_tensor("out", [M, NC_], F32, kind="ExternalOutput")

    with TileContext(nc) as tc, ExitStack() as ctx:
        pool = ctx.enter_context(tc.tile_pool(name="main", bufs=1))

        # ---------------- static tiles ----------------
        w_in_sb = pool.tile([1, 64], F32)
        b_in_sb = pool.tile([64, 1], F32)
        w_h_sb = pool.tile([64, 64], F32)
        b_h_sb = pool.tile([64, 1], F32)
        w_out_sb = pool.tile([64, 1], F32)
        sigvec_sb = pool.tile([P, 1], F32)
        z0_sb = pool.tile([1, NC_], F32)
        shift_sb = [pool.tile([P, P], F32, name=f"shift{d}") for d in range(7)]
        tang_sb = pool.tile([P, Q, NCOLS, NC_], F32)
        idpat_sb = pool.tile([1, NC_ * NCOLS], F32)
        sel_sb = [pool.tile([1, P], F32, name=f"sel{d}") for d in range(7)]
        B = pool.tile([P, Q, 8, 6], F32)
        W = pool.tile([P, Q, NC_], F32)
        Wprev = pool.tile([P, Q, NC_], F32)

        X = pool.tile([P, Q, NCOLS, NC_], F32)
        XS = pool.tile([P, Q, NCOLS, NC_], F32)
        ACC = pool.tile([P, Q, NCOLS, NC_], F32)
        Ks = [pool.tile([P, Q, NCOLS, NC_], F32, name=f"K{j}") for j in range(6)]
        LL = pool.tile([P, Q, NCOLS], F32)
        U = pool.tile([P, Q, NCOLS], F32)
        T0 = pool.tile([P, Q, 1], F32)
        T1 = pool.tile([P, Q, NCOLS - 1], F32)
        T2 = pool.tile([P, Q, NCOLS - 1], F32)
        TMP = pool.tile([P, Q, NCOLS], F32)
        TMPn = pool.tile([P, Q, NCOLS], F32)
        TMPp = pool.tile([P, Q, NCOLS], F32)

        SC = pool.tile([P, Q, NC_, NCOLS], F32)
        SC2 = pool.tile([P, Q, NC_, NCOLS], F32)
        CT = pool.tile([P, Q, NC_, NCOLS], F32)
        AG = pool.tile([P, NC_, NCOLS], F32)
        AG2 = pool.tile([P, NC_, NCOLS], F32)
        EV = pool.tile([P, Q, NC_], F32)

        # ---------------- load constants ----------------
        nc.sync.dma_start(out=w_in_sb, in_=w_in_d[:])
        nc.sync.dma_start(out=b_in_sb, in_=b_in_d[:])
        nc.sync.dma_start(out=w_h_sb, in_=w_h_d[:])
        nc.sync.dma_start(out=b_h_sb, in_=b_h_d[:])
        nc.sync.dma_start(out=w_out_sb, in_=w_out_d[:])
        nc.sync.dma_start(out=sigvec_sb, in_=sigb_d[:])
        nc.sync.dma_start(out=z0_sb, in_=z0_d[:])
        for d in range(7):
            nc.sync.dma_start(out=shift_sb[d], in_=shifts_d[d : d + 1, :, :].squeeze(0))
        nc.sync.dma_start(out=tang_sb.rearrange("p a b c -> p (a b c)"), in_=tang_d[:])
        nc.sync.dma_start(out=idpat_sb, in_=idpat_d[:])
        for d in range(7):
            nc.sync.dma_start(out=sel_sb[d], in_=sel_d[d : d + 1, :, :].squeeze(0))
        nc.sync.dma_start(out=W, in_=winit_d[:])

        # ---------------- Phase A (sharded: each core computes 1/8 of the
        # stage times; pre-sigmoid outputs are AllGathered) ----------------
        CH = NT // 8   # 6144 t-values per core
        NS = CH // 512
        Bflat = B.rearrange("p a b c -> p (a b c)")   # [128, 384]
        OPRE = pool.tile([P, Q * 48], F32)
        with tc.tile_pool(name="phA", bufs=1) as pha, \
             tc.tile_pool(name="psA", bufs=2, space="PSUM") as psA, \
             tc.tile_pool(name="drA", bufs=1, space="DRAM") as dra:
            XH = pha.tile([64, CH], F32)
            TA = pha.tile([64, CH], F32)
            TB = pha.tile([64, CH], F32)
            HH = pha.tile([64, CH], F32)
            tch = pha.tile([1, CH], F32)
            OV = pha.tile([1, CH], F32)
            o_local = dra.tile([1, CH], F32)
            o_all = dra.tile([8, CH], F32)
            nc.sync.dma_start(out=tch, in_=t_stage_d.ap().flatten().unsqueeze(0))

            def softplus_chain(dst):
                nc.scalar.activation(out=TA, in_=XH, func=ACTF.Abs)
                nc.scalar.activation(out=TB, in_=TA, func=ACTF.Exp, scale=-1.0)
                nc.scalar.activation(out=TA, in_=TB, func=ACTF.Ln, bias=1.0)
                nc.scalar.activation(out=TB, in_=XH, func=ACTF.Relu)
                nc.vector.tensor_add(dst, TB, TA)

            for s in range(NS):
                sl = slice(s * 512, (s + 1) * 512)
                p1 = psA.tile([64, 512], F32, name=f"p1_{s}", tag="p1")
                nc.tensor.matmul(p1, w_in_sb, tch[:, sl], start=True, stop=True)
                nc.scalar.activation(out=XH[:, sl], in_=p1, func=ACTF.Identity,
                                     bias=b_in_sb, scale=1.0)
            softplus_chain(HH)
            for s in range(NS):
                sl = slice(s * 512, (s + 1) * 512)
                p2 = psA.tile([64, 512], F32, name=f"p2_{s}", tag="p2")
                nc.tensor.matmul(p2, w_h_sb, HH[:, sl], start=True, stop=True)
                nc.scalar.activation(out=XH[:, sl], in_=p2, func=ACTF.Identity,
                                     bias=b_h_sb, scale=1.0)
            softplus_chain(HH)
            for s in range(NS):
                sl = slice(s * 512, (s + 1) * 512)
                p3 = psA.tile([1, 512], F32, name=f"p3_{s}", tag="p3")
                nc.tensor.matmul(p3, w_out_sb, HH[:, sl], start=True, stop=True)
                nc.scalar.activation(out=OV[:, sl], in_=p3, func=ACTF.Identity)
            nc.sync.dma_start(out=o_local, in_=OV)
            if sim_no_collective:
                # timing stand-in for the AllGather (TimelineSim can't model
                # multi-core collectives): 8 rank-sized DMAs
                for r in range(8):
                    nc.sync.dma_start(out=o_all[r : r + 1, :], in_=o_local)
            else:
                nc.gpsimd.collective_compute(
                    "AllGather", AL.bypass, replica_groups=[list(range(8))],
                    ins=[o_local.opt()], outs=[o_all.opt()])
            nc.sync.dma_start(out=OPRE, in_=o_all.rearrange("a b -> (a b)")
                              .rearrange("(p f) -> p f", p=P))
        nc.scalar.activation(out=Bflat, in_=OPRE, func=ACTF.Sigmoid,
                             bias=sigvec_sb, scale=1e-4)

        psB = ctx.enter_context(tc.tile_pool(name="psB", bufs=1, space="PSUM"))

        # ---------------- helpers ----------------
        def c_sl(t, comp):
            return t[:, :, :, comp : comp + 1].squeeze(3)

        def rhs(src, Kj, b_ap):
            S = c_sl(src, 0)
            E = c_sl(src, 1)
            I = c_sl(src, 2)
            A = c_sl(src, 3)
            nc.vector.scalar_tensor_tensor(out=LL, in0=I, scalar=0.5, in1=A,
                                           op0=AL.mult, op1=AL.add)
            S0 = src[:, :, 0:1, 0:1].squeeze(3)
            LL0 = LL[:, :, 0:1]
            Stan = src[:, :, 1:NCOLS, 0:1].squeeze(3)
            nT = NCOLS - 1
            if b_ap is None:
                nc.vector.tensor_mul(T0, S0, LL0)
                nc.vector.tensor_scalar_mul(U[:, :, 0:1], T0, 0.5)
            else:
                nc.vector.tensor_mul(T0, b_ap, S0)
                nc.vector.tensor_mul(U[:, :, 0:1], T0, LL0)
            nc.vector.tensor_mul(T1, Stan, LL0.broadcast_to([P, Q, nT]))
            nc.vector.tensor_mul(T2, S0.broadcast_to([P, Q, nT]), LL[:, :, 1:NCOLS])
            nc.vector.tensor_add(T1, T1, T2)
            if b_ap is None:
                nc.vector.tensor_scalar_mul(U[:, :, 1:NCOLS], T1, 0.5)
            else:
                nc.vector.tensor_mul(U[:, :, 1:NCOLS], T1,
                                     b_ap.broadcast_to([P, Q, nT]))
            nc.scalar.mul(c_sl(Kj, 0), U, -1.0)
            nc.vector.scalar_tensor_tensor(out=c_sl(Kj, 1), in0=E, scalar=-_f(KKf),
                                           in1=U, op0=AL.mult, op1=AL.add)
            nc.scalar.mul(TMP, E, _f(PKK))
            nc.vector.scalar_tensor_tensor(out=c_sl(Kj, 2), in0=I, scalar=-_f(AAf),
                                           in1=TMP, op0=AL.mult, op1=AL.add)
            nc.scalar.mul(TMPn, A, -_f(IIf))
            nc.vector.scalar_tensor_tensor(out=c_sl(Kj, 3), in0=E, scalar=_f(QKK),
                                           in1=TMPn, op0=AL.mult, op1=AL.add)
            nc.scalar.mul(TMPp, A, _f(IIf))
            nc.vector.scalar_tensor_tensor(out=c_sl(Kj, 4), in0=I, scalar=_f(FAA),
                                           in1=TMPp, op0=AL.mult, op1=AL.add)

        def combine4(dst, right, left_r, left_l, q_dst, q_right, q_left):
            """L1 combine on 4-dim tiles: dst[:, q_dst] = right[:, q_right] o
            (left tile)[:, q_left]."""
            D = dst[:, q_dst, :, :]
            R = right[:, q_right, :, :]
            L = left_l[:, q_left, :, :]
            C = CT[:, q_dst, :, :]
            shp = list(R.shape)
            for k in range(NC_):
                a2 = R[:, :, :, k : k + 1].broadcast_to(shp)
                a1 = L[:, :, k : k + 1, :].broadcast_to(shp)
                if k == 0:
                    nc.vector.tensor_mul(C, a2, a1)
                else:
                    nc.vector.tensor_mul(D, a2, a1)
                    nc.vector.tensor_add(C, C, D)
            nc.vector.tensor_add(C[:, :, :, NC_ : NC_ + 1],
                                 C[:, :, :, NC_ : NC_ + 1],
                                 R[:, :, :, NC_ : NC_ + 1])
            nc.vector.tensor_scalar(out=D, in0=C, scalar1=-CLAMP, scalar2=CLAMP,
                                    op0=AL.max, op1=AL.min)

        def combine3(dst, right, left):
            """L2 combine on 3-dim [P,5,6] tiles over ALL partitions; `left`
            is a PSUM view holding shifted elements (identity for p < d)."""
            D = dst
            R = right
            L = left
            C = CT[:, 0:1, :, :].squeeze(1)
            shp = [P, NC_, NCOLS]
            for k in range(NC_):
                a2 = R[:, :, k : k + 1].broadcast_to(shp)
                a1 = L[:, k : k + 1, :].broadcast_to(shp)
                if k == 0:
                    nc.vector.tensor_mul(C, a2, a1)
                else:
                    nc.vector.tensor_mul(D, a2, a1)
                    nc.vector.tensor_add(C, C, D)
            nc.vector.tensor_add(C[:, :, NC_ : NC_ + 1], C[:, :, NC_ : NC_ + 1],
                                 R[:, :, NC_ : NC_ + 1])
            nc.vector.tensor_scalar(out=D, in0=C, scalar1=-CLAMP, scalar2=CLAMP,
                                    op0=AL.max, op1=AL.min)

        def iteration(fine, it):
            pw = psB.tile([P, NC_], F32, name=f"pw{it}", tag="pw")
            nc.tensor.matmul(pw, shift_sb[0], W[:, 7:8, :].squeeze(1),
                             start=True, stop=True)
            nc.scalar.copy(out=Wprev[:, 1:8, :], in_=W[:, 0:7, :])
            nc.scalar.copy(out=Wprev[:, 0:1, :].squeeze(1), in_=pw)
            nc.scalar.copy(out=Wprev[0:1, 0:1, :].squeeze(1), in_=z0_sb)

            nc.scalar.copy(out=X, in_=tang_sb)
            nc.scalar.copy(out=X[:, :, 0:1, :].squeeze(2), in_=Wprev)

            if fine:
                for i in range(SUBSTEPS):
                    for j in range(6):
                        if j == 0:
                            src = X
                        elif j == 1:
                            nc.vector.scalar_tensor_tensor(
                                out=XS, in0=Ks[0], scalar=_hA(1, 0), in1=X,
                                op0=AL.mult, op1=AL.add)
                            src = XS
                        else:
                            nc.vector.tensor_scalar_mul(ACC, Ks[0], _f(A_TAB[j][0]))
                            for l in range(1, j):
                                nc.vector.scalar_tensor_tensor(
                                    out=ACC, in0=Ks[l], scalar=_f(A_TAB[j][l]),
                                    in1=ACC, op0=AL.mult, op1=AL.add)
                            nc.vector.scalar_tensor_tensor(
                                out=XS, in0=ACC, scalar=float(H), in1=X,
                                op0=AL.mult, op1=AL.add)
                            src = XS
                        b_ap = B[:, :, i : i + 1, j : j + 1].squeeze(2)
                        rhs(src, Ks[j], b_ap)
                    nc.vector.tensor_scalar_mul(ACC, Ks[0], _f(B_TAB[0]))
                    for l in range(1, 6):
                        nc.vector.scalar_tensor_tensor(
                            out=ACC, in0=Ks[l], scalar=_f(B_TAB[l]), in1=ACC,
                            op0=AL.mult, op1=AL.add)
                    nc.vector.scalar_tensor_tensor(out=X, in0=ACC, scalar=float(H),
                                                   in1=X, op0=AL.mult, op1=AL.add)
            else:
                rhs(X, Ks[0], None)
                nc.vector.scalar_tensor_tensor(out=XS, in0=Ks[0], scalar=0.5, in1=X,
                                               op0=AL.mult, op1=AL.add)
                rhs(XS, Ks[1], None)
                nc.vector.tensor_add(X, X, Ks[1])

            nc.scalar.copy(out=SC[:, :, :, 0:NC_],
                           in_=X[:, :, 1:NCOLS, :].transpose([0, 1, 3, 2]))
            nc.vector.tensor_sub(SC[:, :, :, NC_ : NC_ + 1].squeeze(3),
                                 X[:, :, 0:1, :].squeeze(2), W)
            nc.vector.tensor_scalar(out=SC, in0=SC, scalar1=-CLAMP, scalar2=CLAMP,
                                    op0=AL.max, op1=AL.min)

            # L1 over q
            cur, nxt = SC, SC2
            for d in (1, 2, 4):
                combine4(nxt, cur, cur, cur, slice(d, 8), slice(d, 8),
                         slice(0, 8 - d))
                nc.scalar.copy(out=nxt[:, 0:d, :, :], in_=cur[:, 0:d, :, :])
                cur, nxt = nxt, cur
            SCfin = cur

            # L2 over partitions
            nc.scalar.copy(out=AG, in_=SCfin[:, 7:8, :, :].squeeze(1))
            curA, nxtA = AG, AG2
            for lvl, d in enumerate((1, 2, 4, 8, 16, 32, 64)):
                ps = psB.tile([P, NC_ * NCOLS], F32, name=f"ps{it}_{lvl}",
                              tag="ps_shift")
                nc.tensor.matmul(ps, shift_sb[lvl],
                                 curA.rearrange("p a b -> p (a b)"),
                                 start=True, stop=False)
                nc.tensor.matmul(ps, sel_sb[lvl], idpat_sb, start=False, stop=True)
                combine3(nxtA, curA, ps.rearrange("p (a b) -> p a b", a=NC_))
                curA, nxtA = nxtA, curA

            # L3
            ps2 = psB.tile([P, NC_ * NCOLS], F32, name=f"pse{it}", tag="ps_excl")
            nc.tensor.matmul(ps2, shift_sb[0], curA.rearrange("p a b -> p (a b)"),
                             start=True, stop=True)
            ps2v = ps2.rearrange("p (a b) -> p a b", a=NC_)
            for k in range(NC_):
                a = SCfin[:, :, :, k : k + 1].squeeze(3)
                x = ps2v[:, k : k + 1, NC_ : NC_ + 1].broadcast_to([P, Q, NC_])
                if k == 0:
                    nc.vector.tensor_mul(EV, a, x)
                else:
                    nc.vector.tensor_mul(TMP[:, :, 0:NC_], a, x)
                    nc.vector.tensor_add(EV, EV, TMP[:, :, 0:NC_])
            nc.vector.tensor_add(EV, EV, SCfin[:, :, :, NC_ : NC_ + 1].squeeze(3))

            nc.vector.tensor_add(W, W, EV)
            nc.vector.tensor_scalar(out=W, in0=W, scalar1=BOX_LO, scalar2=BOX_HI,
                                    op0=AL.max, op1=AL.min)

        for it in range(N_COARSE):
            iteration(False, it)
        for it in range(N_FINE):
            iteration(True, N_COARSE + it)

        nc.sync.dma_start(out=out_d[0:1, :], in_=z0_sb)
        nc.sync.dma_start(out=out_d[1 : 1 + 127 * 8, :], in_=W[0:127, :, :])
        nc.sync.dma_start(out=out_d[1 + 127 * 8 : M, :], in_=W[127:128, 0:7, :])

    nc.finalize()
    return nc


# ---------------------------------------------------------------------------
# Host side
# ---------------------------------------------------------------------------

def _host_inputs(ts, state_vec, w_in, b_in, w_h, b_h, w_out, b_out, scales):
    ts = np.asarray(ts, np.float32)
    t0 = ts[:-1]
    harr = ((ts[1:] - ts[:-1]) / f32(SUBSTEPS)).astype(f32)
    i_idx = np.arange(SUBSTEPS, dtype=np.float32)
    tsub = (t0[:, None] + i_idx[None, :] * harr[:, None]).astype(f32)
    stage_t = np.empty((1023, 8, 6), np.float32)
    for j in range(6):
        cj_h = (f32(CS[j]) * harr).astype(f32)
        stage_t[:, :, j] = (tsub + cj_h[:, None]).astype(f32)
    t_full = np.empty((M, 8, 6), np.float32)
    t_full[:1023] = stage_t
    t_full[1023] = stage_t[1022]
    t_cores = t_full.reshape(8, NT // 8)   # per-core slices, rank-contiguous

    sv = np.asarray(state_vec, np.float32)
    e = np.exp((sv - sv.max()).astype(f32)).astype(f32)
    smax = (e / e.sum().astype(f32)).astype(f32)
    scales = np.asarray(scales, np.float32)
    y0n = (smax / scales).astype(f32)
    z0 = (y0n * scales).astype(f32)

    shifts = np.zeros((7, P, P), np.float32)
    for lvl, d in enumerate((1, 2, 4, 8, 16, 32, 64)):
        for k in range(P - d):
            shifts[lvl, k, k + d] = 1.0

    tang = np.zeros((P, Q, NCOLS, NC_), np.float32)
    for t in range(NC_):
        tang[:, :, 1 + t, t] = 1.0

    w_init = np.tile(z0, (P, Q, 1)).astype(f32)

    idpat = np.zeros((1, NC_ * NCOLS), np.float32)
    for r in range(NC_):
        idpat[0, r * NCOLS + r] = 1.0
    sel = np.zeros((7, 1, P), np.float32)
    for lvl, d in enumerate((1, 2, 4, 8, 16, 32, 64)):
        sel[lvl, 0, :d] = 1.0

    base = {
        "t_stage": None,  # per-core, filled in kernel()
        "w_in_t": np.ascontiguousarray(np.asarray(w_in, np.float32).T),
        "b_in_v": np.ascontiguousarray(np.asarray(b_in, np.float32)[:, None]),
        "w_h_t": np.ascontiguousarray(np.asarray(w_h, np.float32).T),
        "b_h_v": np.ascontiguousarray(np.asarray(b_h, np.float32)[:, None]),
        "w_out_t": np.ascontiguousarray(np.asarray(w_out, np.float32).T),
        "sig_bias": np.full((P, 1), f32(f32(1e-4) * np.asarray(b_out, np.float32).reshape(-1)[0]),
                            np.float32),
        "z0_row": z0[None, :].copy(),
        "w_init": w_init,
        "tang_init": tang.reshape(P, Q * NCOLS * NC_).copy(),
        "shifts": shifts,
        "idpat": idpat,
        "sel": sel,
    }
    in_maps = []
    for r in range(8):
        m = dict(base)
        m["t_stage"] = np.ascontiguousarray(t_cores[r : r + 1, :])
        in_maps.append(m)
    return in_maps, z0


def kernel(y0_ignored, ts, state_vec, w_in, b_in, w_h, b_h, w_out, b_out, scales):
    if "nc" not in _CACHE:
        _CACHE["nc"] = _build_program()
    nc = _CACHE["nc"]
    in_maps, _ = _host_inputs(ts, state_vec, w_in, b_in, w_h, b_h, w_out, b_out,
                              scales)
    res = run_bass_kernel_spmd(nc, in_maps, list(range(8)))
    return np.asarray(res.results[0]["out"], np.float32)
